# revision 18
# baseline (speedup 1.0000x reference)
"""BERT-CRF Trainium2 kernel: data-parallel over batch across 8 NeuronCores.

Contract: kernel(**inputs) takes the full (unsharded) inputs from
reference.setup_inputs() and returns (score[B] f32, path[B,T] int32).

Design (per core; one sequence per core):
  - activations kept feature-major (H on partitions) so every GEMM contracts
    on the partition dim with natural [K, M] weight layouts;
  - embeddings gathered on-device via indirect DMA from the full table;
  - LayerNorm: token-major for the embedding LN (transposed after), PE-assisted
    (ones-matmul stats + K=1 broadcast matmuls) for the in-layer LNs;
  - softmax without max-subtraction (scores are tiny: |S/8| < ~2), exp on ACT;
  - CRF Viterbi forward scan on-device ([1, 25] max-plus steps on one
    partition); argmax extraction vectorized; backtrace (pure index chasing
    over device-computed argmaxes) on host.
  - fp32 everywhere (weight DMA is the bottleneck at ~28MB/layer/core, so
    fp32 matmul throughput is free; exact fp32 keeps the Viterbi path stable).
"""

import sys

if '/opt/trn_rl_repo' not in sys.path:
    sys.path.insert(0, '/opt/trn_rl_repo')

import numpy as np

B, T, L, H, NH, F, V, K = 8, 128, 12, 768, 12, 3072, 30522, 5
DH = H // NH                     # 64
HK = H // 128                    # 6 chunks of hidden dim
FK = F // 128                    # 24 chunks of ffn dim
NCORES = 8
START, STOP = 3, 4
EPS = 1e-12

_CACHE = {}


# ----------------------------------------------------------------------------
# program builder
# ----------------------------------------------------------------------------

def build_program(n_layers=L, with_viterbi=True, debug_out=None, stage=7):
    """Builds the per-core Bass program (identical on all cores; SPMD over
    different per-core input data).  debug_out: None | 'x0' | 'h' — adds a
    [768, T] fp32 dump of the feature-major activations for bring-up."""
    import concourse.bass as bass
    import concourse.tile as tile
    from concourse import bacc, mybir

    dt = mybir.dt
    f32 = dt.float32
    Alu = mybir.AluOpType
    Act = mybir.ActivationFunctionType

    nc = bacc.Bacc("TRN2", target_bir_lowering=False, debug=False,
                   enable_asserts=True, num_devices=NCORES)

    def din(name, shape, dtype=f32):
        return nc.dram_tensor(name, shape, dtype, kind="ExternalInput").ap()

    def dout(name, shape, dtype=f32):
        return nc.dram_tensor(name, shape, dtype, kind="ExternalOutput").ap()

    # ---- DRAM inputs (host-prepped layouts) ----
    ids = din("ids", (T, 1), dt.uint32)           # token ids, one per partition
    wemb = din("wemb", (V, H))                    # full embedding table
    posplus = din("posplus", (T, H))              # pos_emb + type_emb[0]
    # weights, pre-tiled on host to [p=128, ko, m] DMA-contiguous layout
    wqk = din("wqk", (L, 128, HK, 2 * H))         # Wqkv[:, :1536]
    wv = din("wv", (L, 128, HK, H))               # Wqkv[:, 1536:]
    wo = din("wo", (L, 128, HK, H))
    w1 = din("w1", (L, 128, HK, F))
    w2 = din("w2", (L, 4, 128, HK, H))   # [koq, p, ko-within, m]
    # biases / LN params, feature-major packs [128, chunk, layer]
    bqk_fm = din("bqk_fm", (128, 2 * HK, L))
    bv_rows = din("bv_rows", (L, H))
    bo_fm = din("bo_fm", (128, HK, L))
    b1_fm = din("b1_fm", (128, FK, L))
    b2_fm = din("b2_fm", (128, HK, L))
    ln1s_fm = din("ln1s_fm", (128, HK, L))
    ln1b_fm = din("ln1b_fm", (128, HK, L))
    ln2s_fm = din("ln2s_fm", (128, HK, L))
    ln2b_fm = din("ln2b_fm", (128, HK, L))
    lnes_fm = din("lnes_fm", (128, HK))
    lneb_fm = din("lneb_fm", (128, HK))
    wt_fm = din("wt_fm", (128, HK, K))
    bt_col = din("bt_col", (K, 1))
    identity_in = din("identity_in", (128, 128))
    ones_col_in = din("ones_col_in", (128, 1))
    ones_row_in = din("ones_row_in", (1, 128))
    trans25 = din("trans25", (1, 25))
    init_ld = din("init_ld", (1, K))
    desc5 = din("desc5", (1, K))

    # ---- DRAM outputs ----
    out_ld = dout("out_ld", (1, K))
    out_psis = dout("out_psis", (1, (T - 1) * K))
    out_dbg = dout("out_dbg", (128, HK, T)) if debug_out else None

    with tile.TileContext(nc) as tc:
        from contextlib import ExitStack
        ctx = ExitStack()
        with ctx:
            const = ctx.enter_context(tc.tile_pool(name="const", bufs=1))
            wA = ctx.enter_context(tc.tile_pool(name="wA", bufs=3))
            wB = ctx.enter_context(tc.tile_pool(name="wB", bufs=2))
            acts = ctx.enter_context(tc.tile_pool(name="acts", bufs=1))
            small = ctx.enter_context(tc.tile_pool(name="small", bufs=2))
            vit = ctx.enter_context(tc.tile_pool(name="vit", bufs=1))
            psum_mm = ctx.enter_context(
                tc.tile_pool(name="psum_mm", bufs=3, space="PSUM"))
            psum_tp = ctx.enter_context(
                tc.tile_pool(name="psum_tp", bufs=2, space="PSUM"))
            psum_st = ctx.enter_context(
                tc.tile_pool(name="psum_st", bufs=2, space="PSUM"))
            dram = ctx.enter_context(
                tc.tile_pool(name="dram", bufs=1, space="DRAM"))

            # ---- resident constants ----
            def load_const(ap_in, shape, dtype=f32):
                t = const.tile(list(shape), dtype, tag=ap_in.tensor.name)
                nc.sync.dma_start(t[:], ap_in)
                return t

            identity = load_const(identity_in, (128, 128))
            ones_col = load_const(ones_col_in, (128, 1))
            ones_row = load_const(ones_row_in, (1, 128))
            posplus_sb = load_const(posplus, (T, H))
            bqk_sb = load_const(bqk_fm, (128, 2 * HK, L))
            bo_sb = load_const(bo_fm, (128, HK, L))
            b1_sb = load_const(b1_fm, (128, FK, L))
            b2_sb = load_const(b2_fm, (128, HK, L))
            l1s_sb = load_const(ln1s_fm, (128, HK, L))
            l1b_sb = load_const(ln1b_fm, (128, HK, L))
            l2s_sb = load_const(ln2s_fm, (128, HK, L))
            l2b_sb = load_const(ln2b_fm, (128, HK, L))
            les_sb = load_const(lnes_fm, (128, HK))
            leb_sb = load_const(lneb_fm, (128, HK))
            wt_sb = load_const(wt_fm, (128, HK, K))
            bt_sb = load_const(bt_col, (K, 1))
            trans_sb = load_const(trans25, (1, 25))
            desc_sb = load_const(desc5, (1, K))

            ids_sb = const.tile([T, 1], dt.uint32, tag="ids")
            nc.sync.dma_start(ids_sb[:], ids)

            # =========== embedding ===========
            emb = acts.tile([T, H], f32, tag="emb")
            nc.gpsimd.indirect_dma_start(
                out=emb[:], out_offset=None, in_=wemb[:],
                in_offset=bass.IndirectOffsetOnAxis(ap=ids_sb[:, :1], axis=0))
            nc.vector.tensor_add(out=emb[:], in0=emb[:], in1=posplus_sb[:])

            # token-major LN (per-partition stats)
            def newton_rsqrt(pool, v_ap, shape, n_iter=2):
                """r = 1/sqrt(v) with ACT-sqrt seed + Newton refinement."""
                s0 = pool.tile(list(shape), f32, tag="nr_s0")
                nc.scalar.activation(s0[:], v_ap, Act.Sqrt)
                r = pool.tile(list(shape), f32, tag="nr_r")
                nc.vector.reciprocal(r[:], s0[:])
                for _ in range(n_iter):
                    t1 = pool.tile(list(shape), f32, tag="nr_t1")
                    nc.vector.tensor_tensor(
                        out=t1[:], in0=r[:], in1=r[:], op=Alu.mult)
                    nc.vector.tensor_tensor(
                        out=t1[:], in0=v_ap, in1=t1[:], op=Alu.mult)
                    nc.vector.tensor_scalar(
                        out=t1[:], in0=t1[:], scalar1=-0.5, scalar2=1.5,
                        op0=Alu.mult, op1=Alu.add)
                    rn = pool.tile(list(shape), f32, tag="nr_rn")
                    nc.vector.tensor_tensor(
                        out=rn[:], in0=r[:], in1=t1[:], op=Alu.mult)
                    r = rn
                return r

            s1 = small.tile([T, 1], f32, tag="tm_s1")
            nc.vector.reduce_sum(s1[:], emb[:], axis=mybir.AxisListType.X)
            sq = acts.tile([T, H], f32, tag="emb_sq")
            nc.vector.tensor_tensor(out=sq[:], in0=emb[:], in1=emb[:],
                                    op=Alu.mult)
            s2 = small.tile([T, 1], f32, tag="tm_s2")
            nc.vector.reduce_sum(s2[:], sq[:], axis=mybir.AxisListType.X)
            m = small.tile([T, 1], f32, tag="tm_m")
            nc.vector.tensor_scalar_mul(m[:], s1[:], 1.0 / H)
            var = small.tile([T, 1], f32, tag="tm_var")
            nc.vector.tensor_scalar_mul(var[:], s2[:], 1.0 / H)
            mm_ = small.tile([T, 1], f32, tag="tm_mm")
            nc.vector.tensor_tensor(out=mm_[:], in0=m[:], in1=m[:], op=Alu.mult)
            nc.vector.tensor_tensor(out=var[:], in0=var[:], in1=mm_[:],
                                    op=Alu.subtract)
            nc.vector.tensor_scalar_add(var[:], var[:], EPS)
            r = newton_rsqrt(small, var[:], (T, 1))
            mr = small.tile([T, 1], f32, tag="tm_mr")
            nc.vector.tensor_tensor(out=mr[:], in0=m[:], in1=r[:], op=Alu.mult)
            z = acts.tile([T, H], f32, tag="emb_z")
            nc.vector.tensor_scalar(out=z[:], in0=emb[:], scalar1=r[:],
                                    scalar2=mr[:], op0=Alu.mult,
                                    op1=Alu.subtract)

            # transpose to feature-major + apply emb LN scale/bias
            x = acts.tile([128, HK, T], f32, tag="x")
            for c in range(HK):
                tp = psum_tp.tile([128, 128], f32, tag="tp")
                nc.tensor.transpose(tp[:], z[:, c * 128:(c + 1) * 128],
                                    identity[:])
                nc.vector.tensor_scalar(
                    out=x[:, c, :], in0=tp[:],
                    scalar1=les_sb[:, c:c + 1], scalar2=leb_sb[:, c:c + 1],
                    op0=Alu.mult, op1=Alu.add)

            # =========== transformer layers ===========
            def ln_featmajor(h_in, s_pack, b_pack, li, tag):
                """Feature-major LN via PE stats; returns new [128, HK, T]."""
                # squares
                sqc = acts.tile([128, HK, T], f32, tag=f"sq_{tag}")
                nc.vector.tensor_tensor(out=sqc[:], in0=h_in[:], in1=h_in[:],
                                        op=Alu.mult)
                ps1 = psum_st.tile([1, T], f32, tag="st")
                ps2 = psum_st.tile([1, T], f32, tag="st")
                for c in range(HK):
                    nc.tensor.matmul(ps1[:], lhsT=ones_col[:],
                                     rhs=h_in[:, c, :], start=(c == 0),
                                     stop=(c == HK - 1))
                for c in range(HK):
                    nc.tensor.matmul(ps2[:], lhsT=ones_col[:],
                                     rhs=sqc[:, c, :], start=(c == 0),
                                     stop=(c == HK - 1))
                mrow = small.tile([1, T], f32, tag="fm_m")
                nc.vector.tensor_scalar_mul(mrow[:], ps1[:], 1.0 / H)
                vrow = small.tile([1, T], f32, tag="fm_v")
                nc.vector.tensor_scalar_mul(vrow[:], ps2[:], 1.0 / H)
                m2 = small.tile([1, T], f32, tag="fm_m2")
                nc.vector.tensor_tensor(out=m2[:], in0=mrow[:], in1=mrow[:],
                                        op=Alu.mult)
                nc.vector.tensor_tensor(out=vrow[:], in0=vrow[:], in1=m2[:],
                                        op=Alu.subtract)
                nc.vector.tensor_scalar_add(vrow[:], vrow[:], EPS)
                rrow = newton_rsqrt(small, vrow[:], (1, T))
                mrrow = small.tile([1, T], f32, tag="fm_mr")
                nc.vector.tensor_tensor(out=mrrow[:], in0=mrow[:], in1=rrow[:],
                                        op=Alu.mult)
                # broadcast rows across partitions via K=1 matmul
                bc_r_ps = psum_tp.tile([128, 128], f32, tag="tp")
                nc.tensor.matmul(bc_r_ps[:], lhsT=ones_row[:], rhs=rrow[:],
                                 start=True, stop=True)
                r_bc = small.tile([128, T], f32, tag="fm_rbc")
                nc.vector.tensor_copy(r_bc[:], bc_r_ps[:])
                bc_mr_ps = psum_tp.tile([128, 128], f32, tag="tp")
                nc.tensor.matmul(bc_mr_ps[:], lhsT=ones_row[:], rhs=mrrow[:],
                                 start=True, stop=True)
                mr_bc = small.tile([128, T], f32, tag="fm_mrbc")
                nc.vector.tensor_copy(mr_bc[:], bc_mr_ps[:])
                h_out = acts.tile([128, HK, T], f32, tag=f"hout_{tag}")
                for c in range(HK):
                    tck = small.tile([128, T], f32, tag="fm_t")
                    nc.vector.tensor_tensor(out=tck[:], in0=h_in[:, c, :],
                                            in1=r_bc[:], op=Alu.mult)
                    nc.vector.tensor_tensor(out=tck[:], in0=tck[:],
                                            in1=mr_bc[:], op=Alu.subtract)
                    nc.vector.tensor_scalar(
                        out=h_out[:, c, :], in0=tck[:],
                        scalar1=s_pack[:, c:c + 1, li],
                        scalar2=b_pack[:, c:c + 1, li],
                        op0=Alu.mult, op1=Alu.add)
                return h_out

            for li in range(n_layers):
                # ---- QK^T ----
                if stage < 1:
                    break
                qk = acts.tile([128, 2 * HK, T], f32, tag="qk")
                for mc in range(3):                     # 512-wide chunks
                    wt_ = wA.tile([128, HK, 512], f32, tag="wA")
                    nc.sync.dma_start(
                        wt_[:], wqk[li, :, :, 512 * mc:512 * (mc + 1)])
                    ps = psum_mm.tile([128, 512], f32, tag="mm")
                    for j in range(4):                  # m-groups of 128
                        for c in range(HK):
                            nc.tensor.matmul(
                                ps[:, 128 * j:128 * (j + 1)],
                                lhsT=wt_[:, c, 128 * j:128 * (j + 1)],
                                rhs=x[:, c, :],
                                start=(c == 0), stop=(c == HK - 1))
                        g = 4 * mc + j
                        nc.vector.tensor_scalar_add(
                            qk[:, g, :], ps[:, 128 * j:128 * (j + 1)],
                            bqk_sb[:, g:g + 1, li])

                if stage < 2:
                    continue
                # ---- V (token-major) ----
                v_tm = acts.tile([T, H], f32, tag="v")
                bvrow = small.tile([1, H], f32, tag="bvrow")
                nc.sync.dma_start(bvrow[:], bv_rows[li:li + 1, :])
                for vc in range(2):                     # 384-wide chunks
                    wt_ = wA.tile([128, HK, 384], f32, tag="wA")
                    nc.sync.dma_start(
                        wt_[:], wv[li, :, :, 384 * vc:384 * (vc + 1)])
                    ps = psum_mm.tile([128, 512], f32, tag="mm")
                    for c in range(HK):
                        nc.tensor.matmul(ps[:, :384], lhsT=x[:, c, :],
                                         rhs=wt_[:, c, :],
                                         start=(c == 0), stop=False)
                    nc.tensor.matmul(
                        ps[:, :384], lhsT=ones_row[:],
                        rhs=bvrow[:, 384 * vc:384 * (vc + 1)],
                        start=False, stop=True)
                    nc.vector.tensor_copy(
                        v_tm[:, 384 * vc:384 * (vc + 1)], ps[:, :384])

                if stage < 3:
                    continue
                # ---- attention scores + softmax ----
                p_sb = acts.tile([128, NH, T], f32, tag="p")
                sums = small.tile([128, NH], f32, tag="sums")
                for bank in range(3):
                    ps = psum_mm.tile([128, 512], f32, tag="mm")
                    for hh in range(4):
                        h_ = 4 * bank + hh
                        po = 64 * (h_ % 2)
                        nc.tensor.matmul(
                            ps[:, 128 * hh:128 * (hh + 1)],
                            lhsT=qk[po:po + 64, h_ // 2, :],
                            rhs=qk[po:po + 64, HK + h_ // 2, :],
                            start=True, stop=True)
                        # NB: the PSUM read must not span regions written by
                        # matmuls with different row-group base partitions
                        # (HW crash) — read per head.
                        nc.scalar.activation(
                            p_sb[:, 4 * bank + hh, :],
                            ps[:, 128 * hh:128 * (hh + 1)],
                            Act.Exp, scale=1.0 / np.sqrt(DH))
                    nc.vector.reduce_sum(
                        sums[:, 4 * bank:4 * (bank + 1)],
                        p_sb[:, 4 * bank:4 * (bank + 1), :],
                        axis=mybir.AxisListType.X)
                rec = small.tile([128, NH], f32, tag="rec")
                nc.vector.reciprocal(rec[:], sums[:])
                for bank in range(3):
                    nc.vector.tensor_tensor(
                        out=p_sb[:, 4 * bank:4 * (bank + 1), :],
                        in0=p_sb[:, 4 * bank:4 * (bank + 1), :],
                        in1=rec[:, 4 * bank:4 * (bank + 1), None].to_broadcast(
                            [128, 4, T]),
                        op=Alu.mult)

                if stage < 4:
                    continue
                # ---- P^T and ctx^T ----
                pT = acts.tile([128, NH, T], f32, tag="pT")
                for h_ in range(NH):
                    tp = psum_tp.tile([128, 128], f32, tag="tp")
                    nc.tensor.transpose(tp[:], p_sb[:, h_, :], identity[:])
                    nc.vector.tensor_copy(pT[:, h_, :], tp[:])
                ctx_fm = acts.tile([128, HK, T], f32, tag="ctx")
                for pr in range(HK):
                    cps = psum_tp.tile([128, 128], f32, tag="tp")
                    h0, h1_ = 2 * pr, 2 * pr + 1
                    nc.tensor.matmul(
                        cps[0:64, :], lhsT=v_tm[:, 64 * h0:64 * h0 + 64],
                        rhs=pT[:, h0, :], start=True, stop=True,
                        tile_position=(0, 0))
                    nc.tensor.matmul(
                        cps[64:128, :], lhsT=v_tm[:, 64 * h1_:64 * h1_ + 64],
                        rhs=pT[:, h1_, :], start=True, stop=True,
                        tile_position=(0, 64))
                    nc.vector.tensor_copy(ctx_fm[:, pr, :], cps[:])

                if stage < 5:
                    continue
                # ---- attention out proj + residual + LN1 ----
                h1 = acts.tile([128, HK, T], f32, tag="h1")
                for oc in range(2):
                    wt_ = wA.tile([128, HK, 384], f32, tag="wA")
                    nc.sync.dma_start(
                        wt_[:], wo[li, :, :, 384 * oc:384 * (oc + 1)])
                    ps = psum_mm.tile([128, 512], f32, tag="mm")
                    for j in range(3):
                        for c in range(HK):
                            nc.tensor.matmul(
                                ps[:, 128 * j:128 * (j + 1)],
                                lhsT=wt_[:, c, 128 * j:128 * (j + 1)],
                                rhs=ctx_fm[:, c, :],
                                start=(c == 0), stop=(c == HK - 1))
                        g = 3 * oc + j
                        tb = small.tile([128, T], f32, tag="resid")
                        nc.vector.tensor_scalar_add(
                            tb[:], ps[:, 128 * j:128 * (j + 1)],
                            bo_sb[:, g:g + 1, li])
                        nc.vector.tensor_tensor(
                            out=h1[:, g, :], in0=tb[:], in1=x[:, g, :],
                            op=Alu.add)
                h1n = ln_featmajor(h1, l1s_sb, l1b_sb, li, "ln1")

                x = h1n
                if stage < 6:
                    continue
                # ---- FFN ----
                ff = acts.tile([128, FK, T], f32, tag="ff")
                for mc in range(6):
                    wt_ = wA.tile([128, HK, 512], f32, tag="wA")
                    nc.sync.dma_start(
                        wt_[:], w1[li, :, :, 512 * mc:512 * (mc + 1)])
                    ps = psum_mm.tile([128, 512], f32, tag="mm")
                    for j in range(4):
                        for c in range(HK):
                            nc.tensor.matmul(
                                ps[:, 128 * j:128 * (j + 1)],
                                lhsT=wt_[:, c, 128 * j:128 * (j + 1)],
                                rhs=h1n[:, c, :],
                                start=(c == 0), stop=(c == HK - 1))
                        g = 4 * mc + j
                        # tanh-approx gelu (matches jax.nn.gelu default):
                        # 0.5*u*(1+tanh(sqrt(2/pi)*(u + 0.044715*u^3)))
                        u = small.tile([128, T], f32, tag="gelu_u")
                        nc.vector.tensor_scalar_add(
                            u[:], ps[:, 128 * j:128 * (j + 1)],
                            b1_sb[:, g:g + 1, li])
                        w = small.tile([128, T], f32, tag="gelu_w")
                        nc.vector.tensor_tensor(
                            out=w[:], in0=u[:], in1=u[:], op=Alu.mult)
                        nc.vector.tensor_scalar(
                            out=w[:], in0=w[:], scalar1=0.044715,
                            scalar2=1.0, op0=Alu.mult, op1=Alu.add)
                        nc.vector.tensor_tensor(
                            out=w[:], in0=w[:], in1=u[:], op=Alu.mult)
                        th = small.tile([128, T], f32, tag="gelu_th")
                        nc.scalar.activation(
                            th[:], w[:], Act.Tanh,
                            scale=float(np.sqrt(2.0 / np.pi)))
                        nc.vector.tensor_tensor(
                            out=th[:], in0=th[:], in1=u[:], op=Alu.mult)
                        nc.vector.tensor_tensor(
                            out=th[:], in0=th[:], in1=u[:], op=Alu.add)
                        nc.vector.tensor_scalar_mul(
                            ff[:, g, :], th[:], 0.5)

                if stage < 7:
                    continue
                h2 = acts.tile([128, HK, T], f32, tag="h2")
                h2acc = acts.tile([128, HK, T], f32, tag="h2acc")
                # K=3072 contraction streamed in 4 quarter chunks; partial
                # sums accumulate in SBUF (one open psum group per bank)
                for koq in range(4):
                    wt_ = wB.tile([128, HK, H], f32, tag="wB")
                    nc.sync.dma_start(wt_[:], w2[li, koq])
                    for j in range(6):
                        ps = psum_mm.tile([128, 512], f32, tag="mm")
                        for c in range(HK):
                            nc.tensor.matmul(
                                ps[:, :T],
                                lhsT=wt_[:, c, 128 * j:128 * (j + 1)],
                                rhs=ff[:, HK * koq + c, :],
                                start=(c == 0), stop=(c == HK - 1))
                        if koq == 0:
                            nc.vector.tensor_copy(h2acc[:, j, :], ps[:, :T])
                        else:
                            nc.vector.tensor_tensor(
                                out=h2acc[:, j, :], in0=h2acc[:, j, :],
                                in1=ps[:, :T], op=Alu.add)
                for j in range(6):
                    tb = small.tile([128, T], f32, tag="resid")
                    nc.vector.tensor_scalar_add(
                        tb[:], h2acc[:, j, :], b2_sb[:, j:j + 1, li])
                    nc.vector.tensor_tensor(
                        out=h2[:, j, :], in0=tb[:], in1=h1n[:, j, :],
                        op=Alu.add)
                x = ln_featmajor(h2, l2s_sb, l2b_sb, li, "ln2")

            if debug_out is not None:
                nc.sync.dma_start(out_dbg[:], x[:])

            if with_viterbi:
                # ---- tag projection: feats^T [K, T] ----
                fps = psum_st.tile([K, T], f32, tag="st")
                for c in range(HK):
                    nc.tensor.matmul(fps[:], lhsT=wt_sb[:, c, :],
                                     rhs=x[:, c, :], start=(c == 0),
                                     stop=(c == HK - 1))
                feats_fm = small.tile([K, T], f32, tag="feats")
                nc.vector.tensor_scalar_add(feats_fm[:], fps[:], bt_sb[:])
                # rearrange to token-major flat [1, T*K] via DRAM bounce
                dscratch = dram.tile([K, T], f32)
                nc.sync.dma_start(dscratch[:], feats_fm[:])
                vit_f = vit.tile([1, T * K], f32, tag="vitf")
                nc.sync.dma_start(
                    vit_f[:].rearrange("p (t k) -> p t k", k=K),
                    dscratch[:].rearrange("k t -> t k"))

                # ---- forward scan ----
                ldh = vit.tile([1, T * K], f32, tag="ldh")
                nc.sync.dma_start(ldh[:, 0:K], init_ld)
                trans_v = trans_sb[:].rearrange("p (i j) -> p i j", j=K)
                for t in range(1, T):
                    mb = vit.tile([1, K * K], f32, tag="mb")
                    prev = ldh[:, (t - 1) * K:t * K]
                    nc.vector.tensor_tensor(
                        out=mb[:].rearrange("p (i j) -> p i j", j=K),
                        in0=trans_v,
                        in1=prev[:, None, :].to_broadcast([1, K, K]),
                        op=Alu.add)
                    nc.vector.reduce_max(
                        ldh[:, t * K:(t + 1) * K],
                        mb[:].rearrange("p (i j) -> p i j", j=K),
                        axis=mybir.AxisListType.X)
                    nc.vector.tensor_tensor(
                        out=ldh[:, t * K:(t + 1) * K],
                        in0=ldh[:, t * K:(t + 1) * K],
                        in1=vit_f[:, t * K:(t + 1) * K], op=Alu.add)

                nc.sync.dma_start(out_ld, ldh[:, (T - 1) * K:T * K])

                # ---- vectorized psi extraction ----
                NT = T - 1
                mm_all = vit.tile([1, NT * K * K], f32, tag="mmall")
                mm_v = mm_all[:].rearrange("p (t i j) -> p t i j", i=K, j=K)
                ld_pre = ldh[:, 0:NT * K].rearrange("p (t k) -> p t k", k=K)
                nc.vector.tensor_tensor(
                    out=mm_v,
                    in0=trans_v[:, None, :, :].to_broadcast([1, NT, K, K]),
                    in1=ld_pre[:, :, None, :].to_broadcast([1, NT, K, K]),
                    op=Alu.add)
                best = vit.tile([1, NT * K], f32, tag="best")
                nc.vector.reduce_max(
                    best[:].rearrange("p (t i) -> p t i", i=K),
                    mm_v, axis=mybir.AxisListType.X)
                nc.vector.tensor_tensor(
                    out=mm_v, in0=mm_v,
                    in1=best[:].rearrange("p (t i) -> p t i", i=K)
                        [:, :, :, None].to_broadcast([1, NT, K, K]),
                    op=Alu.is_ge)
                nc.vector.tensor_tensor(
                    out=mm_v, in0=mm_v,
                    in1=desc_sb[:, None, None, :].to_broadcast([1, NT, K, K]),
                    op=Alu.mult)
                psis = vit.tile([1, NT * K], f32, tag="psis")
                nc.vector.reduce_max(
                    psis[:].rearrange("p (t i) -> p t i", i=K),
                    mm_v, axis=mybir.AxisListType.X)
                nc.vector.tensor_scalar(
                    out=psis[:], in0=psis[:], scalar1=-1.0, scalar2=float(K),
                    op0=Alu.mult, op1=Alu.add)
                nc.sync.dma_start(out_psis, psis[:])
            else:
                # keep outputs written so the NEFF binds them
                zz = small.tile([1, K], f32, tag="zz")
                nc.vector.memset(zz[:], 0.0)
                nc.sync.dma_start(out_ld, zz[:])
                zz2 = small.tile([1, (T - 1) * K], f32, tag="zz2")
                nc.vector.memset(zz2[:], 0.0)
                nc.sync.dma_start(out_psis, zz2[:])

    nc.compile()
    return nc


# ----------------------------------------------------------------------------
# host-side input prep
# ----------------------------------------------------------------------------

def prep_in_maps(inputs, n_layers=L):
    f = lambda a: np.ascontiguousarray(np.asarray(a, dtype=np.float32))

    word_emb = f(inputs['word_emb'])
    posplus = f(np.asarray(inputs['pos_emb'], np.float32)
                + np.asarray(inputs['type_emb'], np.float32)[0][None, :])
    Wqkv = f(inputs['Wqkv'])          # [L, H, 3H]
    bqkv = f(inputs['bqkv'])          # [L, 3H]
    Wo = f(inputs['Wo'])
    bo = f(inputs['bo'])
    W1 = f(inputs['W1'])
    b1 = f(inputs['b1'])
    W2 = f(inputs['W2'])
    b2 = f(inputs['b2'])
    ln1s, ln1b = f(inputs['ln1_s']), f(inputs['ln1_b'])
    ln2s, ln2b = f(inputs['ln2_s']), f(inputs['ln2_b'])
    les, leb = f(inputs['emb_ln_s']), f(inputs['emb_ln_b'])
    Wt, bt = f(inputs['Wt']), f(inputs['bt'])
    trans = f(inputs['transitions'])

    def tile_w(w):  # [K_dim, M] -> [128, K_dim//128, M]
        kd, md = w.shape
        return np.ascontiguousarray(
            w.reshape(kd // 128, 128, md).transpose(1, 0, 2))

    wqk = np.stack([tile_w(Wqkv[i, :, :2 * H]) for i in range(L)])
    wv = np.stack([tile_w(Wqkv[i, :, 2 * H:]) for i in range(L)])
    wo_t = np.stack([tile_w(Wo[i]) for i in range(L)])
    w1_t = np.stack([tile_w(W1[i]) for i in range(L)])
    # FFN2 weights: [koq, p, ko-within-quarter, m] so each quarter-K chunk is
    # one fully-contiguous DMA with 3KB partition lines
    w2_t = np.stack([
        np.ascontiguousarray(
            W2[i].reshape(4, HK, 128, H).transpose(0, 2, 1, 3))
        for i in range(L)])

    def fm_pack(a, nchunk):  # [L, nchunk*128] -> [128, nchunk, L]
        return np.ascontiguousarray(
            a.reshape(L, nchunk, 128).transpose(2, 1, 0))

    bqk_fm = fm_pack(bqkv[:, :2 * H], 2 * HK)
    bv_rows = np.ascontiguousarray(bqkv[:, 2 * H:])
    bo_fm = fm_pack(bo, HK)
    b1_fm = fm_pack(b1, FK)
    b2_fm = fm_pack(b2, HK)
    ln1s_fm = fm_pack(ln1s, HK)
    ln1b_fm = fm_pack(ln1b, HK)
    ln2s_fm = fm_pack(ln2s, HK)
    ln2b_fm = fm_pack(ln2b, HK)
    lnes_fm = np.ascontiguousarray(les.reshape(HK, 128).T)
    lneb_fm = np.ascontiguousarray(leb.reshape(HK, 128).T)
    wt_fm = tile_w(Wt)
    bt_col = np.ascontiguousarray(bt.reshape(K, 1))

    shared = dict(
        wemb=word_emb, posplus=posplus, wqk=wqk, wv=wv, wo=wo_t, w1=w1_t,
        w2=w2_t, bqk_fm=bqk_fm, bv_rows=bv_rows, bo_fm=bo_fm, b1_fm=b1_fm,
        b2_fm=b2_fm, ln1s_fm=ln1s_fm, ln1b_fm=ln1b_fm, ln2s_fm=ln2s_fm,
        ln2b_fm=ln2b_fm, lnes_fm=lnes_fm, lneb_fm=lneb_fm, wt_fm=wt_fm,
        bt_col=bt_col,
        identity_in=np.eye(128, dtype=np.float32),
        ones_col_in=np.ones((128, 1), np.float32),
        ones_row_in=np.ones((1, 128), np.float32),
        trans25=np.ascontiguousarray(trans.reshape(1, 25)),
        init_ld=np.array([[-10000.0] * K], np.float32),
        desc5=np.arange(K, 0, -1, dtype=np.float32).reshape(1, K),
    )
    shared['init_ld'][0, START] = 0.0

    sentence = np.asarray(inputs['sentence']).astype(np.uint32)
    in_maps = []
    for c in range(NCORES):
        m = dict(shared)
        m['ids'] = np.ascontiguousarray(sentence[c].reshape(T, 1))
        in_maps.append(m)
    return in_maps


# ----------------------------------------------------------------------------
# entry point
# ----------------------------------------------------------------------------

def kernel(**inputs):
    import time
    from concourse.bass_utils import run_bass_kernel_spmd

    if 'nc' not in _CACHE:
        t0 = time.time()
        _CACHE['nc'] = build_program()
        print(f'[kernel] program built+compiled in {time.time()-t0:.1f}s',
              flush=True)
    nc = _CACHE['nc']

    t0 = time.time()
    in_maps = prep_in_maps(inputs)
    print(f'[kernel] host prep in {time.time()-t0:.1f}s', flush=True)
    t0 = time.time()
    import os
    trace = bool(int(os.environ.get('BERT_CRF_TRACE', '0')))
    res = run_bass_kernel_spmd(nc, in_maps, core_ids=list(range(NCORES)),
                               trace=trace)
    print(f'[kernel] device run in {time.time()-t0:.1f}s', flush=True)
    _CACHE['last_results'] = res

    scores = np.zeros((B,), np.float32)
    paths = np.zeros((B, T), np.int32)
    for c in range(NCORES):
        ld = res.results[c]['out_ld'].reshape(K)
        psis = res.results[c]['out_psis'].reshape(T - 1, K)
        psis_i = psis.astype(np.int32)
        last = int(np.argmax(ld))
        scores[c] = ld[last]
        p = last
        path = np.empty(T, np.int32)
        path[T - 1] = last
        for i in range(T - 2, -1, -1):
            p = psis_i[i, p]
            path[i] = p
        paths[c] = path
    return scores, paths
